# revision 9
# baseline (speedup 1.0000x reference)
"""Peephole LSTM Trainium2 kernel (8 NeuronCores, data-parallel over batch).

Problem: X (64, 1024, 256) f32 -> peephole LSTM, hidden 512, 1024 steps.
Returns (hidden_outputs (64,1024,512), (h_T (64,512), c_T (64,512))).

Per core (batch shard of 8):
  Phase 0: x_g = X @ U_g + b_g for 4 gates as large PE GEMMs (host supplies
    X^T bf16), results stored to DRAM scratch XP (bf16) in scan layout.
  Scan: 1024 sequential steps; gate pre-activations via
    out[M=batch(8), N=512] = hT.T @ W with 4-way PE column tiling
    (tile_position (0,32j)): the 4 hidden-chunks stream weights through
    the PE concurrently (~4x ingest). State tensors live in "layout L":
      tile[partition = 32*j + b, free = g*128 + e], d = 128*j + e
    so every elementwise op is partition-aligned. h^T/c^T (bf16
    stationaries) are rebuilt each step via PE transposes.

Self-contained: nothing read from disk; all shapes hardcoded.
"""

import os
import sys

sys.path.insert(0, "/opt/trn_rl_repo")

import numpy as np
import ml_dtypes
from contextlib import ExitStack

import concourse.bass as bass
import concourse.bacc as bacc
import concourse.tile as tile
from concourse import mybir
from concourse.bass_utils import run_bass_kernel_spmd
from concourse.masks import make_identity

F32 = mybir.dt.float32
BF16 = mybir.dt.bfloat16
AF = mybir.ActivationFunctionType

BATCH = 64
SEQ = int(os.environ.get("LSTM_SEQ", "1024"))
IN = 256
HID = 512
NCORES = 8
BSH = BATCH // NCORES
RING = 8
UNROLL = 8
# gate order in packed free dim: i, f, o, c
_CACHE = {}


def _install_ntff_hook():
    import types
    try:
        import antenv.axon_hooks  # noqa: F401
        return
    except ImportError:
        pass
    try:
        import trn_agent_boot.trn_boot as tb
        mod = types.ModuleType("antenv.axon_hooks")
        hook = tb._ntff_profile_via_ctypes("/opt/axon/libaxon_pjrt.so")
        mod.get_axon_ntff_profile_hook = lambda: hook
        mod.set_axon_ntff_profile_hook = lambda h: None
        sys.modules["antenv.axon_hooks"] = mod
    except Exception:
        pass


def _build(seq):
    nc = bacc.Bacc("TRN2", target_bir_lowering=False, debug=False,
                   num_devices=NCORES)

    xt_d = nc.dram_tensor("xt", [128, 2, BSH * seq], BF16, kind="ExternalInput").ap()
    ua_d = nc.dram_tensor("ua", [128, 2, 4, HID], BF16, kind="ExternalInput").ap()
    br_d = nc.dram_tensor("br", [4 * HID], F32, kind="ExternalInput").ap()
    wa_d = nc.dram_tensor("wa", [128, 4, 4, HID], BF16, kind="ExternalInput").ap()
    pif_d = nc.dram_tensor("pif", [128, 4, 2, HID], BF16, kind="ExternalInput").ap()
    po_d = nc.dram_tensor("po", [128, 4, HID], BF16, kind="ExternalInput").ap()

    hs_d = nc.dram_tensor("hs", [BSH, seq, HID], F32, kind="ExternalOutput").ap()
    ct_d = nc.dram_tensor("ct", [BSH, HID], F32, kind="ExternalOutput").ap()

    # x scratch, padded by RING steps so prefetch never goes OOB
    xp_d = nc.dram_tensor("xp", [seq + RING, 32, HID], BF16).ap()
    XPROW = 32 * HID  # elements per step row

    with tile.TileContext(nc) as tc, ExitStack() as ctx:
        wpool = ctx.enter_context(tc.tile_pool(name="weights", bufs=1))
        state = ctx.enter_context(tc.tile_pool(name="state", bufs=1))

        wa_sb = wpool.tile([128, 4, 4, HID], BF16, tag="wa")
        pif_sb = wpool.tile([128, 4, 2, HID], BF16, tag="pif")
        po_sb = wpool.tile([128, 4, HID], BF16, tag="po")
        nc.sync.dma_start(out=wa_sb, in_=wa_d)
        nc.sync.dma_start(out=pif_sb, in_=pif_d)
        nc.sync.dma_start(out=po_sb, in_=po_d)

        ident = state.tile([128, 128], F32, tag="ident")
        make_identity(nc, ident)

        hT = [state.tile([128, 4, 32], BF16, name=f"hT{p}", tag=f"hT{p}") for p in range(2)]
        cT = [state.tile([128, 4, 32], BF16, name=f"cT{p}", tag=f"cT{p}") for p in range(2)]
        c_l = [state.tile([128, 128], F32, name=f"c{p}", tag=f"c{p}") for p in range(2)]
        for p in range(2):
            nc.vector.memset(hT[p], 0.0)
            nc.vector.memset(cT[p], 0.0)
            nc.vector.memset(c_l[p], 0.0)

        xr = [state.tile([128, HID], BF16, name=f"xr{s}", tag=f"xr{s}") for s in range(RING)]
        for s in range(RING):
            nc.vector.memset(xr[s], 0.0)
        stage = state.tile([128, UNROLL, 128], F32, tag="stage")

        # ---------------- phase 0: input projections ----------------
        with tc.tile_pool(name="p0sb", bufs=2) as p0sb, \
             tc.tile_pool(name="p0ps", bufs=2, space="PSUM") as p0ps, \
             tc.tile_pool(name="p0one", bufs=1) as p0one:
            xt_sb = p0one.tile([128, 2, BSH * seq], BF16, tag="xt")
            nc.sync.dma_start(out=xt_sb, in_=xt_d)
            ua_sb = p0one.tile([128, 2, 4, HID], BF16, tag="ua")
            nc.sync.dma_start(out=ua_sb, in_=ua_d)
            bias_sb = p0one.tile([128, 4 * HID], F32, tag="bias")
            nc.sync.dma_start(
                out=bias_sb,
                in_=bass.AP(tensor=br_d.tensor, offset=br_d.offset,
                            ap=[[0, 128], [1, 4 * HID]]),
            )

            tprb = seq // 128  # row-tiles per batch element
            for r in range(BSH * tprb):
                b, tc0 = r // tprb, (r % tprb) * 128
                ps = p0ps.tile([128, 4 * HID], F32, tag="psproj")
                for g in range(4):
                    for k in range(2):
                        nc.tensor.matmul(
                            ps[:, HID * g : HID * (g + 1)],
                            xt_sb[:, k, 128 * r : 128 * (r + 1)],
                            ua_sb[:, k, g, :],
                            start=(k == 0), stop=(k == 1),
                        )
                xs = p0sb.tile([128, 4 * HID], BF16, tag="xs")
                nc.vector.scalar_tensor_tensor(
                    out=xs, in0=ps, scalar=0.0, in1=bias_sb,
                    op0=mybir.AluOpType.add, op1=mybir.AluOpType.add,
                )
                for j in range(4):
                    dstj = bass.AP(
                        tensor=xp_d.tensor,
                        offset=xp_d.offset + tc0 * XPROW + (j * 8 + b) * HID,
                        ap=[[XPROW, 128], [128, 4], [1, 128]],
                    )
                    nc.sync.dma_start(
                        out=dstj,
                        in_=xs.rearrange("p (g j e) -> p g j e", g=4, j=4)[:, :, j, :],
                    )

        tc.strict_bb_all_engine_barrier()

        # ---------------- scan ----------------
        pre_p = ctx.enter_context(tc.tile_pool(name="pre", bufs=2))
        psW_p = ctx.enter_context(tc.tile_pool(name="psW", bufs=2, space="PSUM"))
        psP_p = ctx.enter_context(tc.tile_pool(name="psP", bufs=2, space="PSUM"))
        psO_p = ctx.enter_context(tc.tile_pool(name="psO", bufs=2, space="PSUM"))
        psT_p = ctx.enter_context(tc.tile_pool(name="psT", bufs=2, space="PSUM"))

        for s in range(RING):
            for j in range(4):
                nc.sync.dma_start(
                    out=xr[s][32 * j : 32 * j + 8, :],
                    in_=xp_d[s, 8 * j : 8 * (j + 1), :],
                )

        def step(iv, s):
            p = s % 2
            hTp, hTc = hT[1 - p], hT[p]
            cTp, cTc = cT[1 - p], cT[p]
            cp, cc = c_l[1 - p], c_l[p]
            x_t = xr[s % RING]

            psW = psW_p.tile([128, HID], F32, tag="psW")
            psP = psP_p.tile([128, 256], F32, tag="psP")
            psO = psO_p.tile([128, 128], F32, tag="psO")

            for j in range(4):
                for k in range(4):
                    nc.tensor.matmul(
                        psW[32 * j : 32 * j + 32, :],
                        hTp[:, k, :],
                        wa_sb[:, k, :, 128 * j : 128 * (j + 1)],
                        start=(k == 0), stop=(k == 3),
                        tile_position=(0, 32 * j),
                    )
            for j in range(4):
                for k in range(4):
                    nc.tensor.matmul(
                        psP[32 * j : 32 * j + 32, :],
                        cTp[:, k, :],
                        pif_sb[:, k, :, 128 * j : 128 * (j + 1)],
                        start=(k == 0), stop=(k == 3),
                        tile_position=(0, 32 * j),
                    )

            pre = pre_p.tile([128, HID], F32, tag="pre")
            nc.vector.tensor_add(pre, psW, x_t)
            tif = pre_p.tile([128, 256], F32, tag="tif")
            nc.vector.tensor_add(tif, pre[:, 0:256], psP)
            sif = pre_p.tile([128, 256], F32, tag="sif")
            nc.scalar.activation(sif, tif, AF.Sigmoid)
            g_t = pre_p.tile([128, 128], F32, tag="g")
            nc.scalar.activation(g_t, pre[:, 384:512], AF.Tanh)
            ig = pre_p.tile([128, 128], F32, tag="ig")
            nc.vector.tensor_mul(ig, sif[:, 0:128], g_t)
            fc = pre_p.tile([128, 128], F32, tag="fc")
            nc.vector.tensor_mul(fc, sif[:, 128:256], cp)
            nc.vector.tensor_add(cc, ig, fc)

            psTc = psT_p.tile([128, 128], F32, tag="psT")
            nc.tensor.transpose(psTc, cc, ident)
            nc.vector.tensor_copy(
                cTc, psTc.rearrange("p (j q) -> p j q", q=32)
            )

            for j in range(4):
                for k in range(4):
                    nc.tensor.matmul(
                        psO[32 * j : 32 * j + 32, :],
                        cTc[:, k, :],
                        po_sb[:, k, 128 * j : 128 * (j + 1)],
                        start=(k == 0), stop=(k == 3),
                        tile_position=(0, 32 * j),
                    )
            to = pre_p.tile([128, 128], F32, tag="to")
            nc.vector.tensor_add(to, pre[:, 256:384], psO)
            o_t = pre_p.tile([128, 128], F32, tag="o")
            nc.scalar.activation(o_t, to, AF.Sigmoid)
            tc_t = pre_p.tile([128, 128], F32, tag="tc")
            nc.scalar.activation(tc_t, cc, AF.Tanh)
            h_sl = stage[:, s, :]
            nc.vector.tensor_mul(h_sl, o_t, tc_t)
            psTh = psT_p.tile([128, 128], F32, tag="psT")
            nc.tensor.transpose(psTh, h_sl, ident)
            nc.vector.tensor_copy(
                hTc, psTh.rearrange("p (j q) -> p j q", q=32)
            )

            # prefetch x for step iv + s + RING into slot (s % RING)
            for j in range(4):
                srcj = bass.AP(
                    tensor=xp_d.tensor,
                    offset=xp_d.offset + iv * XPROW + (s + RING) * XPROW
                    + j * 8 * HID,
                    ap=[[HID, 8], [1, HID]],
                )
                nc.sync.dma_start(out=x_t[32 * j : 32 * j + 8, :], in_=srcj)

        with tc.For_i(0, seq, UNROLL) as iv:
            for s in range(UNROLL):
                step(iv, s)
            # stage out: hs[b, iv + s, 128j + e], one DMA per hidden-chunk j
            for j in range(4):
                dstj = bass.AP(
                    tensor=hs_d.tensor,
                    offset=hs_d.offset + iv * HID + j * 128,
                    ap=[[seq * HID, 8], [HID, UNROLL], [1, 128]],
                )
                nc.sync.dma_start(
                    out=dstj,
                    in_=stage.rearrange("(j q) s e -> j q s e", q=32)[j, 0:8, :, :],
                )

        # final c_T  (last step index seq-1 -> parity (seq-1) % 2)
        pl = (seq - 1) % 2
        for j in range(4):
            dstj = bass.AP(
                tensor=ct_d.tensor, offset=ct_d.offset + j * 128,
                ap=[[HID, 8], [1, 128]],
            )
            nc.sync.dma_start(
                out=dstj, in_=c_l[pl][32 * j : 32 * j + 8, :]
            )

    nc.compile()
    return nc


def _get(seq):
    if seq not in _CACHE:
        _CACHE[seq] = _build(seq)
    return _CACHE[seq]


def _prep_inputs(X, U, W, b, P, seq):
    """Host-side packing for one core's batch shard. Gate order i,f,o,c."""
    bf = ml_dtypes.bfloat16
    U_i, U_f, U_c, U_o = U
    W_i, W_f, W_c, W_o = W
    b_i, b_f, b_c, b_o = b
    P_i, P_f, P_o = P

    # X^T: [8, seq, 256] -> [256, 8*seq] -> [128, 2, 8*seq] (partition-major)
    xt = np.ascontiguousarray(
        X.reshape(BSH * seq, IN).T.reshape(2, 128, BSH * seq).transpose(1, 0, 2)
    ).astype(bf)

    def pack_k(M, nk):  # [K, N] -> [128, nk, N]
        return np.ascontiguousarray(M.reshape(nk, 128, -1).transpose(1, 0, 2))

    ua = np.stack([pack_k(U_i, 2), pack_k(U_f, 2), pack_k(U_o, 2),
                   pack_k(U_c, 2)], axis=2).astype(bf)  # [128, 2, 4, 512]
    wa = np.stack([pack_k(W_i, 4), pack_k(W_f, 4), pack_k(W_o, 4),
                   pack_k(W_c, 4)], axis=2).astype(bf)  # [128, 4, 4, 512]
    pif = np.stack([pack_k(P_i, 4), pack_k(P_f, 4)], axis=2).astype(bf)
    po = pack_k(P_o, 4).astype(bf)  # [128, 4, 512]
    br = np.concatenate([b_i, b_f, b_o, b_c]).astype(np.float32)  # [2048]

    return {"xt": xt, "ua": ua, "br": br, "wa": wa, "pif": pif, "po": po}


def kernel(**inputs):
    _install_ntff_hook()
    X = np.asarray(inputs["X"], dtype=np.float32)
    seq = X.shape[1]
    U = [np.asarray(inputs[k], np.float32) for k in ("U_i", "U_f", "U_c", "U_o")]
    W = [np.asarray(inputs[k], np.float32) for k in ("W_i", "W_f", "W_c", "W_o")]
    b = [np.asarray(inputs[k], np.float32) for k in ("b_i", "b_f", "b_c", "b_o")]
    P = [np.asarray(inputs[k], np.float32) for k in ("P_i", "P_f", "P_o")]

    nc = _get(seq)
    in_maps = []
    for core in range(NCORES):
        Xs = X[core * BSH : (core + 1) * BSH]
        in_maps.append(_prep_inputs(Xs, U, W, b, P, seq))

    res = run_bass_kernel_spmd(nc, in_maps, list(range(NCORES)))
    hs = np.concatenate([res.results[k]["hs"] for k in range(NCORES)], axis=0)
    ct = np.concatenate([res.results[k]["ct"] for k in range(NCORES)], axis=0)
    ht = np.ascontiguousarray(hs[:, -1, :])
    return (hs, (ht, ct))


def run_traced(**inputs):
    """Like kernel() but with NTFF tracing; returns (output, BassKernelResults)."""
    _install_ntff_hook()
    X = np.asarray(inputs["X"], dtype=np.float32)
    seq = X.shape[1]
    U = [np.asarray(inputs[k], np.float32) for k in ("U_i", "U_f", "U_c", "U_o")]
    W = [np.asarray(inputs[k], np.float32) for k in ("W_i", "W_f", "W_c", "W_o")]
    b = [np.asarray(inputs[k], np.float32) for k in ("b_i", "b_f", "b_c", "b_o")]
    P = [np.asarray(inputs[k], np.float32) for k in ("P_i", "P_f", "P_o")]
    nc = _get(seq)
    in_maps = [
        _prep_inputs(X[c * BSH : (c + 1) * BSH], U, W, b, P, seq)
        for c in range(NCORES)
    ]
    res = run_bass_kernel_spmd(nc, in_maps, list(range(NCORES)), trace=True)
    hs = np.concatenate([res.results[k]["hs"] for k in range(NCORES)], axis=0)
    ct = np.concatenate([res.results[k]["ct"] for k in range(NCORES)], axis=0)
    ht = np.ascontiguousarray(hs[:, -1, :])
    return (hs, (ht, ct)), res
